# revision 19
# baseline (speedup 1.0000x reference)
"""Fused multi-head cross-attention with relation branch, sharded over 8 NeuronCores.

Sharding: data-parallel over batch (4) x tensor-parallel over head halves (2).
Core c handles batch c//2, heads [8*(c%2), 8*(c%2)+8). Each core computes its
partial output projection; the host sums the two partials per batch and adds bo.

v2 structure (vs v1):
  - Host pre-packs every input into its device SBUF layout ([128, N] tiles),
    so each tensor loads with ONE large DMA descriptor (packets round-robin
    over all 16 DMA engines, so big descriptors lose no bandwidth). Issue is
    split across the two HWDGE queues (sync + scalar) plus gpsimd for the
    small persistent tensors, with critical-path tensors first.
  - dc-major phase 1: for each dim-chunk dc, project q/k/rk then immediately
    emit that chunk's scores+exp, so the ACT engine (co-critical at ~74us)
    starts ~15us earlier than a tensor-major ordering.
  - Scores for one (lqh, dc, m) are 4 matmuls (2 branches x 2 row-tiled K=64
    head matmuls) into one [128, 2048] PSUM tile, exp'd by a single ACTIVATE
    (the mask bias is shared by both branches).
  - Softmax normalize chain runs in bf16 (2x DVE rate): PSUM->SBUF copies,
    DRAM-roundtrip batched reciprocal, gpsimd partition-broadcasts, fma.
  - The last PV iteration's accumulators live in spool (free after the last
    exp) so the output projection for lqh=0 - emitted AFTER all PV work -
    can run on xpool banks while the final normalize chain drains.
  - yT is written bf16 (host upcasts, sums partials, adds bo in f32).
"""

import math

import numpy as np

B, LQ, LK, D, H = 4, 1024, 1024, 1024, 16
DK = D // H
SCALE = 1.0 / math.sqrt(DK)
N_CORES = 8
HD = D // 2  # local dims per core (8 heads * 64)
# Keys are compacted host-side: only unmasked keys are shipped (padded to LKP
# with dummy rows whose mask bias is -1e9, so exp()=0 -> exact same math).
LKP = 640
NM = LKP // 128  # lk chunks

_CACHE = {}


def _build_program(lkp=LKP):
    import concourse.bacc as bacc
    import concourse.mybir as mybir
    import concourse.tile as tile

    LKP = lkp
    NM = LKP // 128

    f32 = mybir.dt.float32
    bf16 = mybir.dt.bfloat16
    Exp = mybir.ActivationFunctionType.Exp
    Copy = mybir.ActivationFunctionType.Copy
    Add = mybir.AluOpType.add
    Mult = mybir.AluOpType.mult

    nc = bacc.Bacc(
        "TRN2",
        target_bir_lowering=False,
        debug=False,
        enable_asserts=False,
        num_devices=N_CORES,
    )

    # DRAM I/O. Everything arrives pre-packed in its SBUF layout.
    # x tensors: [128, 8*L] with k-chunk k at cols [k*L, (k+1)*L).
    xq_d = nc.dram_tensor("xq", [128, 8 * LQ], bf16, kind="ExternalInput").ap()
    xk_d = nc.dram_tensor("xk", [128, 8 * LKP], bf16, kind="ExternalInput").ap()
    xr_d = nc.dram_tensor("xr", [128, 8 * LKP], bf16, kind="ExternalInput").ap()
    xv_d = nc.dram_tensor("xv", [128, 8 * LKP], bf16, kind="ExternalInput").ap()
    # transposed-proj weights: [128, 4*8*128]: col = dc*1024 + k*128 + c
    wq_d = nc.dram_tensor("wq", [128, 4096], bf16, kind="ExternalInput").ap()
    wk_d = nc.dram_tensor("wk", [128, 4096], bf16, kind="ExternalInput").ap()
    wrk_d = nc.dram_tensor("wrk", [128, 4096], bf16, kind="ExternalInput").ap()
    # natural-proj weights: [128, 8*512]: col = k*512 + c
    wv_d = nc.dram_tensor("wv", [128, 4096], bf16, kind="ExternalInput").ap()
    wrv_d = nc.dram_tensor("wrv", [128, 4096], bf16, kind="ExternalInput").ap()
    # output-proj weights: [128, 4*1024]: col = dc*1024 + c
    wo_d = nc.dram_tensor("wo", [128, 4096], bf16, kind="ExternalInput").ap()
    bq_pc = nc.dram_tensor("bq_pc", [128, 4], f32, kind="ExternalInput").ap()
    bk_pc = nc.dram_tensor("bk_pc", [128, 4], f32, kind="ExternalInput").ap()
    brk_pc = nc.dram_tensor("brk_pc", [128, 4], f32, kind="ExternalInput").ap()
    bv_bc = nc.dram_tensor("bv_bc", [128, HD], f32, kind="ExternalInput").ap()
    brv_bc = nc.dram_tensor("brv_bc", [128, HD], f32, kind="ExternalInput").ap()
    maskb = nc.dram_tensor("maskb", [128, NM], f32, kind="ExternalInput").ap()
    yT = nc.dram_tensor("yT", [D, LQ], bf16, kind="ExternalOutput").ap()
    scr1 = nc.dram_tensor("scr1", [8, 2048], bf16, kind="Internal").ap()
    scr2 = nc.dram_tensor("scr2", [8, 2048], bf16, kind="Internal").ap()

    with nc.allow_low_precision(
        reason="bf16 normalize pipeline by design; matmul PSUM acc stays f32"
    ), tile.TileContext(nc) as tc:
        from contextlib import ExitStack

        with ExitStack() as ctx:
            # ---- persistent SBUF tensors (whole-program lifetime) ----
            persist = ctx.enter_context(tc.tile_pool(name="persist", bufs=1))
            qT_sb = persist.tile([128, 4 * LQ], bf16, tag="qT")
            kT_sb = persist.tile([128, 4 * LKP], bf16, tag="kT")
            rkT_sb = persist.tile([128, 4 * LKP], bf16, tag="rkT")
            v_sb = persist.tile([128, NM * 8 * 65], bf16, tag="v")
            rv_sb = persist.tile([128, NM * 8 * 65], bf16, tag="rv")
            xf_sb = persist.tile([128, 4 * LQ], bf16, tag="xf")
            wo_sb = persist.tile([128, 4096], bf16, tag="wo")
            maskb_sb = persist.tile([128, NM], f32, tag="maskb")
            bq_sb = persist.tile([128, 4], f32, tag="bq")
            bk_sb = persist.tile([128, 4], f32, tag="bk")
            brk_sb = persist.tile([128, 4], f32, tag="brk")
            bv_sb = persist.tile([128, HD], f32, tag="bv")
            brv_sb = persist.tile([128, HD], f32, tag="brv")

            v4 = v_sb[:].rearrange("p (m h c) -> p m h c", m=NM, h=8, c=65)
            rv4 = rv_sb[:].rearrange("p (m h c) -> p m h c", m=NM, h=8, c=65)
            nc.vector.memset(v4[:, :, :, 64:65], 1.0)
            nc.vector.memset(rv4[:, :, :, 64:65], 1.0)

            # Score PSUM: 2 bufs x [128,1024] so exp(i) overlaps the score
            # matmuls of tile i+1 (a single buffer serializes the ACT engine
            # behind a semaphore round-trip per tile). ppool = exp lookahead.
            spool = ctx.enter_context(tc.tile_pool(name="spool", bufs=2, space="PSUM"))
            ppool = ctx.enter_context(tc.tile_pool(name="ppool", bufs=32))
            p_tiles = {}

            def emit_score_pair(lqh, dc, m, br, kt):
                qsl = slice(1024 * dc + 512 * lqh, 1024 * dc + 512 * lqh + 512)
                ksl = slice(LKP * dc + 128 * m, LKP * dc + 128 * m + 128)
                s = spool.tile([128, 1024], f32, tag="spool", name="s")
                nc.tensor.matmul(s[:, 0:512], kt[0:64, ksl], qT_sb[0:64, qsl])
                nc.tensor.matmul(s[:, 512:1024], kt[64:128, ksl], qT_sb[64:128, qsl])
                p = ppool.tile([128, 1024], bf16, tag="ppool", name="p")
                nc.scalar.activation(
                    p[:], s[:], Exp, bias=maskb_sb[:, m : m + 1], scale=SCALE
                )
                p_tiles[(lqh, dc, m, br)] = p

            def emit_scores(lqh, dc):
                for m in range(NM):
                    for br, kt in ((0, kT_sb), (1, rkT_sb)):
                        emit_score_pair(lqh, dc, m, br, kt)

            # ---------------- Phase 1: dc-major q/k/rk + scores(0) ----------
            with ExitStack() as ph1:
                ppsum = ph1.enter_context(
                    tc.tile_pool(name="ppsum", bufs=2, space="PSUM")
                )
                # input tensors live only through phase 1; phase-2 pools
                # reuse their SBUF space
                xin = ph1.enter_context(tc.tile_pool(name="xin", bufs=1))
                xq_sb = xin.tile([128, 8 * LQ], bf16, tag="xq")
                xk_sb = xin.tile([128, 8 * LKP], bf16, tag="xk")
                xr_sb = xin.tile([128, 8 * LKP], bf16, tag="xr")
                xv_sb = xin.tile([128, 8 * LKP], bf16, tag="xv")
                wq_sb = xin.tile([128, 4096], bf16, tag="wq")
                wk_sb = xin.tile([128, 4096], bf16, tag="wk")
                wrk_sb = xin.tile([128, 4096], bf16, tag="wrk")
                wv_sb = xin.tile([128, 4096], bf16, tag="wv")
                wrv_sb = xin.tile([128, 4096], bf16, tag="wrv")

                # ---- input DMAs ----
                # A single in-flight descriptor stream tops out well below the
                # 358GB/s aggregate, so each big tensor is split into quarters
                # interleaved across both HWDGE queues (sync + scalar), in
                # consumption order: xq/wq(dc0) -> xk/wk(dc0) -> xr/wrk(dc0)
                # -> remaining weights -> v-branch inputs.
                def split4(dst, src, n):
                    q = n // 4
                    for j in range(4):
                        eng = nc.sync if j % 2 == 0 else nc.scalar
                        eng.dma_start(
                            out=dst[:, j * q : (j + 1) * q],
                            in_=src[:, j * q : (j + 1) * q],
                        )

                nc.scalar.dma_start(out=wq_sb[:, 0:1024], in_=wq_d[:, 0:1024])
                split4(xq_sb, xq_d, 8 * LQ)
                split4(xk_sb, xk_d, 8 * LKP)
                nc.scalar.dma_start(out=wk_sb[:, 0:1024], in_=wk_d[:, 0:1024])
                split4(xr_sb, xr_d, 8 * LKP)
                nc.sync.dma_start(out=wrk_sb[:, 0:1024], in_=wrk_d[:, 0:1024])
                nc.scalar.dma_start(out=wq_sb[:, 1024:4096], in_=wq_d[:, 1024:4096])
                nc.sync.dma_start(out=wk_sb[:, 1024:4096], in_=wk_d[:, 1024:4096])
                nc.scalar.dma_start(out=wrk_sb[:, 1024:4096], in_=wrk_d[:, 1024:4096])
                split4(xv_sb, xv_d, 8 * LKP)
                nc.sync.dma_start(out=wv_sb[:], in_=wv_d)
                nc.scalar.dma_start(out=wrv_sb[:], in_=wrv_d)
                nc.sync.dma_start(out=wo_sb[:], in_=wo_d)
                nc.scalar.dma_start(out=bv_sb[:], in_=bv_bc)
                nc.sync.dma_start(out=brv_sb[:], in_=brv_bc)
                # gpsimd (software DGE): tiny persistent tensors
                nc.gpsimd.dma_start(out=maskb_sb[:], in_=maskb)
                nc.gpsimd.dma_start(out=bq_sb[:], in_=bq_pc)
                nc.gpsimd.dma_start(out=bk_sb[:], in_=bk_pc)
                nc.gpsimd.dma_start(out=brk_sb[:], in_=brk_pc)

                def tproj(dc, x_sb, w_sb, b_sb, out_sb, LL):
                    nsl = [slice(a, min(a + 512, LL)) for a in range(0, LL, 512)]
                    ps = ppsum.tile([128, LL], f32, tag="ppsum")
                    for k in range(8):
                        wsl = w_sb[:, 1024 * dc + 128 * k : 1024 * dc + 128 * k + 128]
                        for sl in nsl:
                            nc.tensor.matmul(
                                ps[:, sl],
                                wsl,
                                x_sb[:, LL * k + sl.start : LL * k + sl.stop],
                                start=(k == 0),
                                stop=(k == 7),
                            )
                    nc.vector.tensor_scalar(
                        out=out_sb[:, LL * dc : LL * dc + LL],
                        in0=ps[:],
                        scalar1=b_sb[:, dc : dc + 1],
                        scalar2=None,
                        op0=Add,
                    )

                for dc in range(4):
                    tproj(dc, xq_sb, wq_sb, bq_sb, qT_sb, LQ)
                    tproj(dc, xk_sb, wk_sb, bk_sb, kT_sb, LKP)
                    tproj(dc, xr_sb, wrk_sb, brk_sb, rkT_sb, LKP)
                    emit_scores(0, dc)

                # Natural-orientation projections for v / rv come BEFORE the
                # lqh=1 scores: PV consumption must start as early as possible
                # or the exp stream stalls on ppool backpressure.
                for x_sb, w_sb, b_sb, out4 in (
                    (xv_sb, wv_sb, bv_sb, v4),
                    (xr_sb, wrv_sb, brv_sb, rv4),
                ):
                    for m in range(NM):
                        ps = ppsum.tile([128, 512], f32, tag="ppsum")
                        for k in range(8):
                            nc.tensor.matmul(
                                ps[:],
                                x_sb[:, LKP * k + 128 * m : LKP * k + 128 * m + 128],
                                w_sb[:, 512 * k : 512 * k + 512],
                                start=(k == 0),
                                stop=(k == 7),
                            )
                        nc.vector.tensor_tensor(
                            out=out4[:, m, :, 0:64],
                            in0=ps[:].rearrange("p (h c) -> p h c", h=8, c=64),
                            in1=b_sb[:].rearrange("p (h c) -> p h c", h=8, c=64),
                            op=Add,
                        )

                for dc in range(4):
                    emit_scores(1, dc)

            # -------- Phase 2: PV accumulation, normalize, output projection --
            # The normalize chain is software-pipelined one iteration deep so
            # its DMA round-trip latency never blocks the in-order DVE queue:
            #   front(it): PV matmuls + PSUM->SBUF evictions + Z-row DMAs
            #   finish(it-1): broadcast-multiply + add -> xf
            #   mid(it): reciprocal + scatter + partition-broadcast issues
            with ExitStack() as ph2:
                xpool = ph2.enter_context(
                    tc.tile_pool(name="xpool", bufs=4, space="PSUM")
                )
                xsb = ph2.enter_context(tc.tile_pool(name="xsb", bufs=6))
                sgp = ph2.enter_context(tc.tile_pool(name="sgp", bufs=2))
                bcp = ph2.enter_context(tc.tile_pool(name="bcp", bufs=8))
                ysb = ph2.enter_context(tc.tile_pool(name="ysb", bufs=4))

                def pv_front(lqh, dc, use_spool, last=False):
                    it = 2 * dc + lqh
                    xacc = {}
                    if use_spool:
                        w2a = spool.tile([128, 1024], f32, tag="spool", name="xacca")
                        w2b = spool.tile([128, 1024], f32, tag="spool", name="xaccb")
                        for j, (br, hs) in enumerate(
                            [(0, 0), (0, 1), (1, 0), (1, 1)]
                        ):
                            w2 = w2a if j < 2 else w2b
                            xacc[(br, hs)] = w2[:, 512 * (j % 2) : 512 * (j % 2) + 512]
                    else:
                        for br in range(2):
                            for hs in range(2):
                                xacc[(br, hs)] = xpool.tile(
                                    [65, 512], f32, tag="xpool", name=f"xacc{br}{hs}"
                                )[:, :]
                    for m in range(NM):
                        for br, vv in ((0, v4), (1, rv4)):
                            pt = p_tiles[(lqh, dc, m, br)]
                            for hs in range(2):
                                nc.tensor.matmul(
                                    xacc[(br, hs)][0:65, :],
                                    vv[:, m, 2 * dc + hs, :],
                                    pt[:, 512 * hs : 512 * hs + 512],
                                    start=(m == 0),
                                    stop=(m == NM - 1),
                                )
                    # Evict accumulators (bf16) so the PSUM banks free fast;
                    # row 64 carries the softmax denominators. For the last
                    # iteration the chain is latency-critical: exps are done,
                    # so half the copies go to the idle ACT engine.
                    xs_all = xsb.tile([65, 2048], bf16, tag="xsall", bufs=3)
                    xs = {}
                    for j, (br, hs) in enumerate([(0, 0), (1, 0), (0, 1), (1, 1)]):
                        sl = xs_all[:, 512 * j : 512 * j + 512]
                        if last and j >= 2:
                            nc.scalar.activation(sl, xacc[(br, hs)][0:65, :], Copy)
                        else:
                            nc.vector.tensor_copy(out=sl, in_=xacc[(br, hs)][0:65, :])
                        xs[(br, hs)] = sl
                    # Respread the [1,2048] denominator row over 128 DVE lanes
                    # with an SBUF->SBUF DMA (no DRAM round-trip).
                    sg = sgp.tile([128, 16], bf16, tag="sgp")
                    nc.sync.dma_start(out=sg[:], in_=xs_all[64:65, :])
                    return {"it": it, "lqh": lqh, "dc": dc, "xs": xs, "sg": sg,
                            "last": last}

                def pv_mid(st):
                    it = st["it"]
                    nc.vector.reciprocal(st["sg"][:], st["sg"][:])
                    zrow = sgp.tile([1, 2048], bf16, tag="zrow")
                    nc.sync.dma_start(out=zrow[:], in_=st["sg"][:])
                    if st["last"]:
                        # chain-latency-critical: run half the broadcasts via
                        # the DRAM path in parallel with gpsimd's two
                        nc.scalar.dma_start(out=scr2[it, :], in_=st["sg"][:])
                    bc = {}
                    for j in range(4):
                        t = bcp.tile([64, 512], bf16, tag="bcp", name=f"bc{j}")
                        if st["last"] and j >= 2:
                            nc.sync.dma_start(
                                out=t[:],
                                in_=scr2[it : it + 1, 512 * j : 512 * j + 512]
                                .partition_broadcast(64)[:, 0, :],
                            )
                        else:
                            nc.gpsimd.partition_broadcast(
                                t[:], zrow[0:1, 512 * j : 512 * j + 512], channels=64
                            )
                        bc[j] = t
                    st["bc"] = bc

                def pv_finish(st):
                    lqh, dc, xs, bc = st["lqh"], st["dc"], st["xs"], st["bc"]
                    last = st["last"]
                    for hs in range(2):
                        jv, jr = 2 * hs, 2 * hs + 1
                        t1 = xsb.tile([65, 512], bf16, tag="xsb")
                        eng1 = nc.gpsimd if last else nc.vector
                        eng1.tensor_tensor(
                            out=t1[0:64, :], in0=xs[(0, hs)][0:64, :],
                            in1=bc[jv][:], op=Mult,
                        )
                        t2 = xsb.tile([65, 512], bf16, tag="xsb")
                        nc.vector.tensor_tensor(
                            out=t2[0:64, :], in0=xs[(1, hs)][0:64, :],
                            in1=bc[jr][:], op=Mult,
                        )
                        xf_slice = slice(
                            1024 * dc + 512 * lqh, 1024 * dc + 512 * lqh + 512
                        )
                        if hs == 0:
                            nc.vector.tensor_tensor(
                                out=xf_sb[0:64, xf_slice], in0=t1[0:64, :],
                                in1=t2[0:64, :], op=Add,
                            )
                        else:
                            t3 = xsb.tile([65, 512], bf16, tag="xsb")
                            eng3 = nc.gpsimd if last else nc.vector
                            eng3.tensor_tensor(
                                out=t3[0:64, :], in0=t1[0:64, :], in1=t2[0:64, :],
                                op=Add,
                            )
                            nc.sync.dma_start(
                                out=xf_sb[64:128, xf_slice], in_=t3[0:64, :]
                            )

                def y_evict(ps, ot, lqh, qno):
                    # PSUM->SBUF eviction alternating ACT/DVE (both idle by
                    # now), DMA alternating between the two HWDGE queues.
                    y = ysb.tile([128, 512], bf16, tag="ysb")
                    if qno % 2 == 0:
                        nc.scalar.activation(y[:], ps[:], Copy)
                    else:
                        nc.vector.tensor_copy(out=y[:], in_=ps[:])
                    eng = nc.sync if qno % 2 == 0 else nc.scalar
                    eng.dma_start(
                        out=yT[128 * ot : 128 * ot + 128, 512 * lqh : 512 * lqh + 512],
                        in_=y[:],
                    )

                def emit_outproj0():
                    for ot in range(8):
                        ps = xpool.tile([128, 512], f32, tag="xpool", name=f"psy{ot}")
                        for dc in range(4):
                            nc.tensor.matmul(
                                ps[:],
                                wo_sb[:, 1024 * dc + 128 * ot : 1024 * dc + 128 * ot + 128],
                                xf_sb[:, 1024 * dc : 1024 * dc + 512],
                                start=(dc == 0),
                                stop=(dc == 3),
                            )
                        y_evict(ps, ot, 0, ot)

                def emit_outproj1():
                    # dc-outer with all 8 accumulators live (4 xpool banks + 4
                    # spool halves). dc=3 is further split into K=64 halves so
                    # only the hs=1 half waits for the final xf shift-DMA.
                    pss = []
                    for i in range(4):
                        pss.append(
                            xpool.tile([128, 512], f32, tag="xpool", name=f"psw{i}")
                        )
                    w3a = spool.tile([128, 1024], f32, tag="spool", name="psw3a")
                    w3b = spool.tile([128, 1024], f32, tag="spool", name="psw3b")
                    for w in (w3a, w3b):
                        pss.append(w[:, 0:512])
                        pss.append(w[:, 512:1024])
                    for dc in range(3):
                        for ot in range(8):
                            nc.tensor.matmul(
                                pss[ot][0:128, :],
                                wo_sb[:, 1024 * dc + 128 * ot : 1024 * dc + 128 * ot + 128],
                                xf_sb[:, 1024 * dc + 512 : 1024 * dc + 1024],
                                start=(dc == 0),
                                stop=False,
                            )
                    for hs in range(2):
                        psl = slice(64 * hs, 64 * hs + 64)
                        for ot in range(8):
                            nc.tensor.matmul(
                                pss[ot][0:128, :],
                                wo_sb[psl, 3072 + 128 * ot : 3072 + 128 * ot + 128],
                                xf_sb[psl, 3584:4096],
                                start=False,
                                stop=(hs == 1),
                            )
                    for ot in range(8):
                        y_evict(pss[ot], ot, 1, ot + 1)

                prev = None
                for lqh in range(2):
                    for dc in range(4):
                        last = lqh == 1 and dc == 3
                        st = pv_front(lqh, dc, use_spool=last, last=last)
                        if prev is not None:
                            pv_finish(prev)
                        pv_mid(st)
                        prev = st
                # outproj(0) has no dependency on the last normalize chain, so
                # the PE stays busy while that chain drains.
                emit_outproj0()
                pv_finish(prev)
                emit_outproj1()

    nc.compile()
    return nc


def _get_program(lkp=LKP):
    if lkp not in _CACHE:
        _CACHE[lkp] = _build_program(lkp)
    return _CACHE[lkp]


def _bf16(arr):
    import ml_dtypes

    return np.ascontiguousarray(np.asarray(arr, dtype=np.float32).astype(ml_dtypes.bfloat16))


def _pack_chunks(mat_t, nk, L):
    """[nk*128, L] -> [128, nk*L] with chunk k at cols [k*L, (k+1)*L)."""
    return mat_t.reshape(nk, 128, L).transpose(1, 0, 2).reshape(128, nk * L)


def _shard_inputs(inputs, lkp=LKP):
    q = np.ascontiguousarray(inputs["query"], dtype=np.float32)
    k = np.ascontiguousarray(inputs["key"], dtype=np.float32)
    v = np.ascontiguousarray(inputs["value"], dtype=np.float32)
    wr = np.ascontiguousarray(inputs["weak_rela"], dtype=np.float32)
    mask = np.asarray(inputs["mask"])

    def t_weight(W, hsl):
        # Wt [D, HD] -> [128, 4096] with col = dc*1024 + k*128 + c
        Wt = np.asarray(W, dtype=np.float32)[hsl, :].T
        return Wt.reshape(8, 128, 4, 128).transpose(1, 2, 0, 3).reshape(128, 4096)

    def n_weight(W, hsl):
        # Wt [D, HD] -> [128, 4096] with col = k*512 + c
        Wt = np.asarray(W, dtype=np.float32)[hsl, :].T
        return _pack_chunks(Wt, 8, 512)

    in_maps = []
    for c in range(N_CORES):
        b, hh = divmod(c, 2)
        hsl = slice(HD * hh, HD * hh + HD)
        idx = np.nonzero(mask[b, 0])[0]
        nv = len(idx)
        assert nv <= lkp
        pidx = np.concatenate([idx, np.zeros(lkp - nv, dtype=idx.dtype)])
        bias = np.full(lkp, -1.0e9, np.float32)
        bias[:nv] = 0.0
        mb = np.ascontiguousarray(bias.reshape(lkp // 128, 128).T)
        kc, vc, wrc = k[b][pidx], v[b][pidx], wr[b][pidx]
        # wo: [HD, D] -> [128, 4096] with col = dc*1024 + c
        woT = np.asarray(inputs["Wo"], dtype=np.float32)[:, hsl].T
        wo_p = _pack_chunks(woT, 4, 1024)
        m = {
            "xq": _bf16(_pack_chunks(q[b].T, 8, LQ)),
            "xk": _bf16(_pack_chunks(kc.T, 8, lkp)),
            "xr": _bf16(_pack_chunks(wrc.T, 8, lkp)),
            "xv": _bf16(_pack_chunks(vc.T, 8, lkp)),
            "wq": _bf16(t_weight(inputs["Wq"], hsl)),
            "wk": _bf16(t_weight(inputs["Wk"], hsl)),
            "wrk": _bf16(t_weight(inputs["Wrk"], hsl)),
            "wv": _bf16(n_weight(inputs["Wv"], hsl)),
            "wrv": _bf16(n_weight(inputs["Wrv"], hsl)),
            "wo": _bf16(wo_p),
            "bq_pc": np.asarray(inputs["bq"][hsl]).reshape(4, 128).T.astype(np.float32),
            "bk_pc": np.asarray(inputs["bk"][hsl]).reshape(4, 128).T.astype(np.float32),
            "brk_pc": np.asarray(inputs["brk"][hsl]).reshape(4, 128).T.astype(np.float32),
            "bv_bc": np.broadcast_to(inputs["bv"][hsl], (128, HD)).astype(np.float32),
            "brv_bc": np.broadcast_to(inputs["brv"][hsl], (128, HD)).astype(np.float32),
            "maskb": mb,
        }
        in_maps.append({k2: np.ascontiguousarray(v2) for k2, v2 in m.items()})
    return in_maps


def run_on_hw(inputs, trace=False, **kw):
    from concourse.bass_utils import run_bass_kernel_spmd

    mask = np.asarray(inputs["mask"])
    max_valid = max(int(mask[b, 0].sum()) for b in range(B))
    lkp = max(LKP, ((max_valid + 127) // 128) * 128)
    nc = _get_program(lkp)
    in_maps = _shard_inputs(inputs, lkp)
    res = run_bass_kernel_spmd(
        nc, in_maps, core_ids=list(range(N_CORES)), trace=trace, **kw
    )
    bo = np.asarray(inputs["bo"], dtype=np.float32)
    outs = []
    for b in range(B):
        yt = res.results[2 * b]["yT"].astype(np.float32) + res.results[
            2 * b + 1
        ]["yT"].astype(np.float32)
        outs.append(yt.T + bo)
    out = np.stack(outs).astype(np.float32)
    return out, res


def kernel(**inputs):
    out, _ = run_on_hw(inputs)
    return out
